# revision 37
# baseline (speedup 1.0000x reference)
"""Trainium2 Bass kernel for location-sensitive attention (Tacotron-style).

Reference computation (per batch b):
    pq  = tanh(ahs[b] @ Wq + bq)                  # [ADIM]
    pm  = tanh(mem[b] @ Wm + bm)                  # [T, ADIM]
    cv  = awc[b] @ conv_w[KSIZE//2]               # SAME conv on width-1 input
    paw = tanh(cv @ Wloc)                         # [ADIM]
    e   = tanh(pq + pm + paw) @ Vw (+ Vb)         # [T]; Vb cancels in softmax
    aw  = softmax(e)                              # [T]
    ctx = aw[:, None] * mem[b]                    # [T, ENC]

Sharded data-parallel over batch across 8 NeuronCores; weights replicated.
"""

import numpy as np

B, T, ENC, QDIM, ADIM, NFILT, KSIZE = 64, 1024, 512, 1024, 128, 32, 31
NCORES = 8
BL = B // NCORES          # batches per core
TC = T // 128             # 8 t-chunks of 128
EC = ENC // 128           # 4 e-chunks of 128
QC = QDIM // 128          # 8 q-chunks
NCH = T // 512            # 2 moving-dim chunks for N<=512 matmuls
GROUPS = [2, 2, 2, 1, 1]  # batch group sizes (softmax granularity); the
                          # small tail groups shorten the post-chain after
                          # the last memory load
GRP = len(GROUPS)
GB = max(GROUPS)          # widest group (vwz layout)
MEM_DMA_SPLIT = 4         # dma_starts per batch for the big memory stream

_cache = {}


def _build():
    from concourse import bacc, tile, mybir

    f32 = mybir.dt.float32
    f32r = mybir.dt.float32r
    bf16 = mybir.dt.bfloat16
    Act = mybir.ActivationFunctionType

    nc = bacc.Bacc("TRN2", target_bir_lowering=False, debug=False,
                   num_devices=NCORES)

    mem_d = nc.dram_tensor("memory", [BL, T, ENC], f32, kind="ExternalInput")
    ahs_d = nc.dram_tensor("ahs", [BL, QDIM], f32, kind="ExternalInput")
    awc_d = nc.dram_tensor("awc", [BL, T], f32, kind="ExternalInput")
    wq_d = nc.dram_tensor("wq", [QDIM, ADIM], f32, kind="ExternalInput")
    bq_d = nc.dram_tensor("bq", [ADIM, 1], f32, kind="ExternalInput")
    wm_d = nc.dram_tensor("wm", [ENC, ADIM], f32, kind="ExternalInput")
    bm_d = nc.dram_tensor("bm", [ADIM, 1], f32, kind="ExternalInput")
    wc_d = nc.dram_tensor("wc", [T, NFILT], f32, kind="ExternalInput")
    wloc_d = nc.dram_tensor("wloc", [NFILT, ADIM], f32, kind="ExternalInput")
    vw_d = nc.dram_tensor("vw", [ADIM, 1], f32, kind="ExternalInput")
    id_d = nc.dram_tensor("ident", [128, 128], f32, kind="ExternalInput")

    ctx_d = nc.dram_tensor("out_ctx", [BL, T, ENC], f32, kind="ExternalOutput")
    aw_d = nc.dram_tensor("out_aw", [BL, T], f32, kind="ExternalOutput")

    with tile.TileContext(nc) as tc:
        with (
            tc.tile_pool(name="wpool", bufs=1) as wpool,
            tc.tile_pool(name="mem", bufs=BL) as mempool,
            tc.tile_pool(name="memT", bufs=8) as mtpool,
            tc.tile_pool(name="ein", bufs=4) as epool,
            tc.tile_pool(name="soft", bufs=2) as spool,
            tc.tile_pool(name="psT", bufs=2, space="PSUM") as psT,
            tc.tile_pool(name="psPM", bufs=2, space="PSUM") as psPM,
            tc.tile_pool(name="psE", bufs=2, space="PSUM") as psE,
        ):
            # ---- resident weights / constants -------------------------------
            wq_t = wpool.tile([128, QDIM], f32, tag="wq")       # chunk k at [:, 128k:]
            nc.sync.dma_start(wq_t[:].rearrange("p (k a) -> p k a", k=QC),
                              wq_d[:].rearrange("(k p) a -> p k a", p=128))
            wm_t = wpool.tile([128, EC * ADIM], f32, tag="wm")
            nc.sync.dma_start(wm_t[:].rearrange("p (k a) -> p k a", k=EC),
                              wm_d[:].rearrange("(k p) a -> p k a", p=128))
            wc_t = wpool.tile([128, TC * NFILT], f32, tag="wc")
            nc.sync.dma_start(wc_t[:].rearrange("p (k a) -> p k a", k=TC),
                              wc_d[:].rearrange("(k p) a -> p k a", p=128))
            wloc_t = wpool.tile([NFILT, ADIM], f32, tag="wloc")
            nc.scalar.dma_start(wloc_t[:], wloc_d[:])
            vw_t = wpool.tile([ADIM, 1], f32, tag="vw")
            nc.scalar.dma_start(vw_t[:], vw_d[:])
            bq_t = wpool.tile([ADIM, 1], f32, tag="bq")
            nc.scalar.dma_start(bq_t[:], bq_d[:])
            bm_t = wpool.tile([ADIM, 1], f32, tag="bm")
            nc.scalar.dma_start(bm_t[:], bm_d[:])
            id_t = wpool.tile([128, 128], f32, tag="ident")
            nc.scalar.dma_start(id_t[:], id_d[:])
            idr = id_t[:]

            id_b = wpool.tile([128, 128], bf16, tag="id_b")
            nc.vector.tensor_copy(id_b[:], id_t[:])

            # f32r/bf16 copies of matmul weight operands (rounding casts)
            wq_r = wpool.tile([128, QDIM], f32r, tag="wq_r")
            nc.vector.tensor_copy(wq_r[:], wq_t[:])
            wm_b = wpool.tile([128, EC * ADIM], bf16, tag="wm_b")
            nc.vector.tensor_copy(wm_b[:], wm_t[:])
            wc_r = wpool.tile([128, TC * NFILT], f32r, tag="wc_r")
            nc.vector.tensor_copy(wc_r[:], wc_t[:])
            wloc_r = wpool.tile([NFILT, ADIM], f32r, tag="wloc_r")
            nc.vector.tensor_copy(wloc_r[:], wloc_t[:])

            ahs_t = wpool.tile([BL, QDIM], f32, tag="ahs")
            nc.scalar.dma_start(ahs_t[:], ahs_d[:])
            awc_t = wpool.tile([BL, T], f32, tag="awc")
            nc.scalar.dma_start(awc_t[:], awc_d[:])

            # memory loads dispatch after the small weights so the weight
            # data (needed by all compute) isn't starved behind 17MB of
            # memory traffic on the sync HWDGE FIFO.
            mem_t = {}
            rows = T // MEM_DMA_SPLIT
            cols = rows * ENC // 128
            for b in range(BL):
                mem_t[b] = mempool.tile([128, TC * ENC], f32, tag="mem",
                                        name=f"mem_{b}")
                for s in range(MEM_DMA_SPLIT):
                    nc.sync.dma_start(
                        mem_t[b][:, s * cols : (s + 1) * cols]
                        .rearrange("p (tc e) -> p tc e", e=ENC),
                        mem_d[b][s * rows : (s + 1) * rows, :]
                        .rearrange("(tc p) e -> p tc e", p=128))

            # Vw replicated into per-batch-slot columns of zero matrices:
            # vwz[:, GB*j + j] = Vw  -> lhsT for batch j writes energy row j.
            vwz_f = wpool.tile([ADIM, GB * GB], f32, tag="vwz_f")
            nc.vector.memset(vwz_f[:], 0.0)
            for j in range(GB):
                nc.vector.tensor_copy(vwz_f[:, GB * j + j : GB * j + j + 1],
                                      vw_t[:])
            vwz = wpool.tile([ADIM, GB * GB], f32r, tag="vwz")
            nc.vector.tensor_copy(vwz[:], vwz_f[:])

            # ---- phase 0: per-batch bias = tanh(pq) + tanh(paw) + bm --------
            # transpose ahs/awc so QDIM/T land on partitions
            ahsT = wpool.tile([128, QC * BL], f32r, tag="ahsT")
            for k in range(QC):
                ps = psT.tile([128, 512], f32, tag="psT")
                nc.tensor.transpose(
                    ps[:, 0:BL], ahs_t[0:BL, 128 * k : 128 * (k + 1)],
                    idr[0:BL, 0:BL])
                nc.vector.tensor_copy(ahsT[:, BL * k : BL * (k + 1)],
                                      ps[:, 0:BL])
            awcT = wpool.tile([128, TC * BL], f32r, tag="awcT")
            for k in range(TC):
                ps = psT.tile([128, 512], f32, tag="psT")
                nc.tensor.transpose(
                    ps[:, 0:BL], awc_t[0:BL, 128 * k : 128 * (k + 1)],
                    idr[0:BL, 0:BL])
                nc.vector.tensor_copy(awcT[:, BL * k : BL * (k + 1)],
                                      ps[:, 0:BL])

            pq_ps = psPM.tile([ADIM, BL], f32, tag="psPM")
            for k in range(QC):
                nc.tensor.matmul(
                    pq_ps[:], wq_r[:, 128 * k : 128 * (k + 1)],
                    ahsT[:, BL * k : BL * (k + 1)],
                    start=(k == 0), stop=(k == QC - 1))
            pq_sb = wpool.tile([ADIM, BL], f32, tag="pq_sb")
            nc.scalar.activation(pq_sb[:], pq_ps[:], Act.Tanh, bias=bq_t[:])

            cv_ps = psPM.tile([NFILT, BL], f32, tag="psPM")
            for k in range(TC):
                nc.tensor.matmul(
                    cv_ps[:], wc_r[:, NFILT * k : NFILT * (k + 1)],
                    awcT[:, BL * k : BL * (k + 1)],
                    start=(k == 0), stop=(k == TC - 1))
            cv_sb = wpool.tile([NFILT, BL], f32r, tag="cv_sb")
            nc.scalar.activation(cv_sb[:], cv_ps[:], Act.Tanh)

            paw_ps = psPM.tile([ADIM, BL], f32, tag="psPM")
            nc.tensor.matmul(paw_ps[:], wloc_r[:], cv_sb[:])
            bias_all = wpool.tile([ADIM, BL], f32, tag="bias_all")
            nc.scalar.activation(bias_all[:], paw_ps[:], Act.Tanh)
            nc.vector.tensor_tensor(bias_all[:], bias_all[:], pq_sb[:],
                                    op=mybir.AluOpType.add)

            awt = wpool.tile([128, BL * TC], f32, tag="awt")

            # ---- main pipeline, software-pipelined emission order ----------
            # Engines execute their instruction streams in order, so post(g)
            # (softmax -> aw transpose -> scale -> store) is emitted AFTER
            # compute(g+1); its dependencies resolve while compute(g+1) runs
            # and no engine stalls at group boundaries.
            E_t = {}

            def compute(g):
                off = sum(GROUPS[:g])
                sz = GROUPS[g]
                E = psE.tile([GB, NCH * 512], f32, tag="psE", name=f"E_{g}")
                E_t[g] = E
                for j in range(sz):
                    b = off + j
                    for q in range(NCH):
                        mb = mtpool.tile([128, 4 * ENC], bf16, tag="mem_b",
                                         name=f"mb_{b}_{q}", bufs=3)
                        nc.vector.tensor_copy(
                            mb[:], mem_t[b][:, 2048 * q : 2048 * (q + 1)])
                        mts = []
                        for ecp in range(EC // 2):
                            ps = psT.tile([128, 1024], bf16, tag="psT",
                                          name=f"ps_{b}_{q}_{ecp}")
                            for ec2 in range(2):
                                ec = 2 * ecp + ec2
                                for t4 in range(4):
                                    nc.tensor.transpose(
                                        ps[:, 512 * ec2 + 128 * t4 :
                                           512 * ec2 + 128 * (t4 + 1)],
                                        mb[:, ENC * t4 + 128 * ec :
                                           ENC * t4 + 128 * (ec + 1)],
                                        id_b[:, :])
                            mt = mtpool.tile([128, 1024], bf16, tag="memT",
                                             name=f"mt_{b}_{q}_{ecp}", bufs=4)
                            nc.vector.tensor_copy(mt[:], ps[:])
                            mts.append(mt)
                        pm_ps = psPM.tile([128, 512], f32, tag="psPM",
                                          name=f"pm_ps_{b}_{q}")
                        for ec in range(EC):
                            nc.tensor.matmul(
                                pm_ps[:],
                                wm_b[:, 128 * ec : 128 * (ec + 1)],
                                mts[ec // 2][:, 512 * (ec % 2) :
                                             512 * (ec % 2 + 1)],
                                start=(ec == 0), stop=(ec == EC - 1))
                        pm_sb = epool.tile([128, 512], f32, tag="pm_sb",
                                           name=f"pm_sb_{b}_{q}")
                        nc.scalar.activation(pm_sb[:], pm_ps[:], Act.Tanh,
                                             bias=bm_t[:])
                        e_in = epool.tile([128, 512], f32r, tag="ein",
                                          name=f"e_in_{b}_{q}")
                        nc.scalar.activation(e_in[:], pm_sb[:], Act.Tanh,
                                             bias=bias_all[:, b : b + 1])
                        nc.tensor.matmul(
                            E[0:sz, 512 * q : 512 * (q + 1)],
                            vwz[:, GB * j : GB * j + sz], e_in[:],
                            start=(j == 0), stop=(j == sz - 1),
                            skip_group_check=True)

            def post(g):
                off = sum(GROUPS[:g])
                sz = GROUPS[g]
                E = E_t[g]
                rmax = spool.tile([GB, 1], f32, tag="rmax", name=f"rmax_{g}")
                r1 = spool.tile([GB, 1], f32, tag="r1", name=f"r1_{g}")
                nc.vector.tensor_reduce(rmax[0:sz, :], E[0:sz, 0:512],
                                        mybir.AxisListType.X,
                                        mybir.AluOpType.max)
                nc.vector.tensor_reduce(r1[0:sz, :], E[0:sz, 512:1024],
                                        mybir.AxisListType.X,
                                        mybir.AluOpType.max)
                nc.vector.tensor_tensor(rmax[0:sz, :], rmax[0:sz, :],
                                        r1[0:sz, :],
                                        op=mybir.AluOpType.max)
                nmax = spool.tile([GB, 1], f32, tag="nmax", name=f"nmax_{g}")
                nc.vector.tensor_scalar_mul(nmax[0:sz, :], rmax[0:sz, :], -1.0)
                ex = spool.tile([GB, T], f32, tag="ex", name=f"ex_{g}")
                s0 = spool.tile([GB, 1], f32, tag="s0", name=f"s0_{g}")
                s1 = spool.tile([GB, 1], f32, tag="s1", name=f"s1_{g}")
                nc.scalar.activation(ex[0:sz, 0:512], E[0:sz, 0:512], Act.Exp,
                                     bias=nmax[0:sz, :], accum_out=s0[0:sz, :])
                nc.scalar.activation(ex[0:sz, 512:1024], E[0:sz, 512:1024],
                                     Act.Exp,
                                     bias=nmax[0:sz, :], accum_out=s1[0:sz, :])
                nc.vector.tensor_tensor(s0[0:sz, :], s0[0:sz, :], s1[0:sz, :],
                                        op=mybir.AluOpType.add)
                rs = spool.tile([GB, 1], f32, tag="rs", name=f"rs_{g}")
                nc.vector.reciprocal(rs[0:sz, :], s0[0:sz, :])
                aw_sb = spool.tile([GB, T], f32, tag="aw_sb", name=f"aw_{g}")
                nc.vector.tensor_scalar_mul(aw_sb[0:sz, :], ex[0:sz, :],
                                            rs[0:sz, :])
                nc.gpsimd.dma_start(aw_d[off : off + sz, :], aw_sb[0:sz, :])

                coff = off * TC
                for tcc in range(TC):
                    ps = psT.tile([128, 512], f32, tag="psT",
                                  name=f"awt_ps_{g}_{tcc}")
                    nc.tensor.transpose(
                        ps[:, 0:sz],
                        aw_sb[0:sz, 128 * tcc : 128 * (tcc + 1)],
                        idr[0:sz, 0:sz])
                    nc.vector.tensor_copy(
                        awt[:, coff + sz * tcc : coff + sz * (tcc + 1)],
                        ps[:, 0:sz])

                for j in range(sz):
                    b = off + j
                    for tcc in range(TC):
                        col = coff + sz * tcc + j
                        if tcc < 4:
                            nc.scalar.mul(
                                mem_t[b][:, ENC * tcc : ENC * (tcc + 1)],
                                mem_t[b][:, ENC * tcc : ENC * (tcc + 1)],
                                awt[:, col : col + 1])
                        else:
                            nc.vector.tensor_scalar_mul(
                                mem_t[b][:, ENC * tcc : ENC * (tcc + 1)],
                                mem_t[b][:, ENC * tcc : ENC * (tcc + 1)],
                                awt[:, col : col + 1])
                    for s in range(MEM_DMA_SPLIT):
                        nc.gpsimd.dma_start(
                            ctx_d[b][s * rows : (s + 1) * rows, :]
                            .rearrange("(tc p) e -> p tc e", p=128),
                            mem_t[b][:, s * cols : (s + 1) * cols]
                            .rearrange("p (tc e) -> p tc e", e=ENC))

            compute(0)
            for g in range(1, GRP):
                compute(g)
                post(g - 1)
            post(GRP - 1)

    nc.compile()
    return nc


def _get_nc():
    if "nc" not in _cache:
        _cache["nc"] = _build()
    return _cache["nc"]


def kernel(attention_hidden_state, memory, attention_weights_cat,
           Wq, bq, Wm, bm, conv_w, Wloc, Vw, Vb):
    from concourse.bass_utils import run_bass_kernel_spmd

    mem = np.ascontiguousarray(np.asarray(memory, np.float32))
    ahs = np.ascontiguousarray(
        np.asarray(attention_hidden_state, np.float32).reshape(B, QDIM))
    awc = np.ascontiguousarray(np.asarray(attention_weights_cat, np.float32))
    wq = np.ascontiguousarray(np.asarray(Wq, np.float32))
    wm = np.ascontiguousarray(np.asarray(Wm, np.float32))
    wc = np.ascontiguousarray(np.asarray(conv_w, np.float32)[KSIZE // 2])
    wloc = np.ascontiguousarray(np.asarray(Wloc, np.float32))
    vw = np.ascontiguousarray(np.asarray(Vw, np.float32).reshape(ADIM, 1))
    bqv = np.ascontiguousarray(np.asarray(bq, np.float32).reshape(ADIM, 1))
    bmv = np.ascontiguousarray(np.asarray(bm, np.float32).reshape(ADIM, 1))
    ident = np.eye(128, dtype=np.float32)
    # Vb shifts every energy equally; softmax (and so both outputs) is
    # invariant to it.

    nc = _get_nc()
    in_maps = []
    for c in range(NCORES):
        sl = slice(c * BL, (c + 1) * BL)
        in_maps.append({
            "memory": mem[sl], "ahs": ahs[sl], "awc": awc[sl],
            "wq": wq, "bq": bqv, "wm": wm, "bm": bmv,
            "wc": wc, "wloc": wloc, "vw": vw, "ident": ident,
        })
    res = run_bass_kernel_spmd(nc, in_maps, list(range(NCORES))).results
    ctx = np.concatenate([res[c]["out_ctx"] for c in range(NCORES)], axis=0)
    aw = np.concatenate([res[c]["out_aw"] for c in range(NCORES)], axis=0)
    return ctx, aw


# revision 38
# speedup vs baseline: 1.0413x; 1.0413x over previous
"""Trainium2 Bass kernel for location-sensitive attention (Tacotron-style).

Reference computation (per batch b):
    pq  = tanh(ahs[b] @ Wq + bq)                  # [ADIM]
    pm  = tanh(mem[b] @ Wm + bm)                  # [T, ADIM]
    cv  = awc[b] @ conv_w[KSIZE//2]               # SAME conv on width-1 input
    paw = tanh(cv @ Wloc)                         # [ADIM]
    e   = tanh(pq + pm + paw) @ Vw (+ Vb)         # [T]; Vb cancels in softmax
    aw  = softmax(e)                              # [T]
    ctx = aw[:, None] * mem[b]                    # [T, ENC]

Sharded data-parallel over batch across 8 NeuronCores; weights replicated.
"""

import numpy as np

B, T, ENC, QDIM, ADIM, NFILT, KSIZE = 64, 1024, 512, 1024, 128, 32, 31
NCORES = 8
BL = B // NCORES          # batches per core
TC = T // 128             # 8 t-chunks of 128
EC = ENC // 128           # 4 e-chunks of 128
QC = QDIM // 128          # 8 q-chunks
NCH = T // 512            # 2 moving-dim chunks for N<=512 matmuls
GROUPS = [2, 2, 2, 1, 1]  # batch group sizes (softmax granularity); the
                          # small tail groups shorten the post-chain after
                          # the last memory load
GRP = len(GROUPS)
GB = max(GROUPS)          # widest group (vwz layout)
MEM_DMA_SPLIT = 4         # dma_starts per batch for the big memory stream

_cache = {}


def _build():
    from concourse import bacc, tile, mybir

    f32 = mybir.dt.float32
    f32r = mybir.dt.float32r
    bf16 = mybir.dt.bfloat16
    Act = mybir.ActivationFunctionType

    nc = bacc.Bacc("TRN2", target_bir_lowering=False, debug=False,
                   num_devices=NCORES)

    mem_d = nc.dram_tensor("memory", [BL, T, ENC], f32, kind="ExternalInput")
    ahs_d = nc.dram_tensor("ahs", [BL, QDIM], f32, kind="ExternalInput")
    awc_d = nc.dram_tensor("awc", [BL, T], f32, kind="ExternalInput")
    wq_d = nc.dram_tensor("wq", [QDIM, ADIM], f32, kind="ExternalInput")
    bq_d = nc.dram_tensor("bq", [ADIM, 1], f32, kind="ExternalInput")
    wm_d = nc.dram_tensor("wm", [ENC, ADIM], f32, kind="ExternalInput")
    bm_d = nc.dram_tensor("bm", [ADIM, 1], f32, kind="ExternalInput")
    wc_d = nc.dram_tensor("wc", [T, NFILT], f32, kind="ExternalInput")
    wloc_d = nc.dram_tensor("wloc", [NFILT, ADIM], f32, kind="ExternalInput")
    vw_d = nc.dram_tensor("vw", [ADIM, 1], f32, kind="ExternalInput")
    id_d = nc.dram_tensor("ident", [128, 128], f32, kind="ExternalInput")

    ctx_d = nc.dram_tensor("out_ctx", [BL, T, ENC], f32, kind="ExternalOutput")
    aw_d = nc.dram_tensor("out_aw", [BL, T], f32, kind="ExternalOutput")

    with tile.TileContext(nc) as tc:
        with (
            tc.tile_pool(name="wpool", bufs=1) as wpool,
            tc.tile_pool(name="mem", bufs=BL) as mempool,
            tc.tile_pool(name="memT", bufs=8) as mtpool,
            tc.tile_pool(name="ein", bufs=4) as epool,
            tc.tile_pool(name="soft", bufs=2) as spool,
            tc.tile_pool(name="psT", bufs=2, space="PSUM") as psT,
            tc.tile_pool(name="psPM", bufs=2, space="PSUM") as psPM,
            tc.tile_pool(name="psE", bufs=2, space="PSUM") as psE,
        ):
            # ---- resident weights / constants -------------------------------
            wq_t = wpool.tile([128, QDIM], f32, tag="wq")       # chunk k at [:, 128k:]
            nc.sync.dma_start(wq_t[:].rearrange("p (k a) -> p k a", k=QC),
                              wq_d[:].rearrange("(k p) a -> p k a", p=128))
            wm_t = wpool.tile([128, EC * ADIM], f32, tag="wm")
            nc.sync.dma_start(wm_t[:].rearrange("p (k a) -> p k a", k=EC),
                              wm_d[:].rearrange("(k p) a -> p k a", p=128))
            wc_t = wpool.tile([128, TC * NFILT], f32, tag="wc")
            nc.sync.dma_start(wc_t[:].rearrange("p (k a) -> p k a", k=TC),
                              wc_d[:].rearrange("(k p) a -> p k a", p=128))
            wloc_t = wpool.tile([NFILT, ADIM], f32, tag="wloc")
            nc.scalar.dma_start(wloc_t[:], wloc_d[:])
            vw_t = wpool.tile([ADIM, 1], f32, tag="vw")
            nc.scalar.dma_start(vw_t[:], vw_d[:])
            bq_t = wpool.tile([ADIM, 1], f32, tag="bq")
            nc.scalar.dma_start(bq_t[:], bq_d[:])
            bm_t = wpool.tile([ADIM, 1], f32, tag="bm")
            nc.scalar.dma_start(bm_t[:], bm_d[:])
            id_t = wpool.tile([128, 128], f32, tag="ident")
            nc.scalar.dma_start(id_t[:], id_d[:])
            idr = id_t[:]

            id_b = wpool.tile([128, 128], bf16, tag="id_b")
            nc.vector.tensor_copy(id_b[:], id_t[:])

            # f32r/bf16 copies of matmul weight operands (rounding casts)
            wq_r = wpool.tile([128, QDIM], f32r, tag="wq_r")
            nc.vector.tensor_copy(wq_r[:], wq_t[:])
            wm_b = wpool.tile([128, EC * ADIM], bf16, tag="wm_b")
            nc.vector.tensor_copy(wm_b[:], wm_t[:])
            wc_r = wpool.tile([128, TC * NFILT], f32r, tag="wc_r")
            nc.vector.tensor_copy(wc_r[:], wc_t[:])
            wloc_r = wpool.tile([NFILT, ADIM], f32r, tag="wloc_r")
            nc.vector.tensor_copy(wloc_r[:], wloc_t[:])

            ahs_t = wpool.tile([BL, QDIM], f32, tag="ahs")
            nc.scalar.dma_start(ahs_t[:], ahs_d[:])
            awc_t = wpool.tile([BL, T], f32, tag="awc")
            nc.scalar.dma_start(awc_t[:], awc_d[:])

            # memory loads dispatch after the small weights so the weight
            # data (needed by all compute) isn't starved behind 17MB of
            # memory traffic on the sync HWDGE FIFO.
            mem_t = {}
            rows = T // MEM_DMA_SPLIT
            cols = rows * ENC // 128
            for b in range(BL):
                mem_t[b] = mempool.tile([128, TC * ENC], f32, tag="mem",
                                        name=f"mem_{b}")
                for s in range(MEM_DMA_SPLIT):
                    nc.sync.dma_start(
                        mem_t[b][:, s * cols : (s + 1) * cols]
                        .rearrange("p (tc e) -> p tc e", e=ENC),
                        mem_d[b][s * rows : (s + 1) * rows, :]
                        .rearrange("(tc p) e -> p tc e", p=128))

            # Vw replicated into per-batch-slot columns of zero matrices:
            # vwz[:, GB*j + j] = Vw  -> lhsT for batch j writes energy row j.
            vwz_f = wpool.tile([ADIM, GB * GB], f32, tag="vwz_f")
            nc.vector.memset(vwz_f[:], 0.0)
            for j in range(GB):
                nc.vector.tensor_copy(vwz_f[:, GB * j + j : GB * j + j + 1],
                                      vw_t[:])
            vwz = wpool.tile([ADIM, GB * GB], f32r, tag="vwz")
            nc.vector.tensor_copy(vwz[:], vwz_f[:])

            # ---- phase 0: per-batch bias = tanh(pq) + tanh(paw) + bm --------
            # transpose ahs/awc so QDIM/T land on partitions
            ahsT = wpool.tile([128, QC * BL], f32r, tag="ahsT")
            for k in range(QC):
                ps = psT.tile([128, 512], f32, tag="psT")
                nc.tensor.transpose(
                    ps[:, 0:BL], ahs_t[0:BL, 128 * k : 128 * (k + 1)],
                    idr[0:BL, 0:BL])
                nc.vector.tensor_copy(ahsT[:, BL * k : BL * (k + 1)],
                                      ps[:, 0:BL])
            awcT = wpool.tile([128, TC * BL], f32r, tag="awcT")
            for k in range(TC):
                ps = psT.tile([128, 512], f32, tag="psT")
                nc.tensor.transpose(
                    ps[:, 0:BL], awc_t[0:BL, 128 * k : 128 * (k + 1)],
                    idr[0:BL, 0:BL])
                nc.vector.tensor_copy(awcT[:, BL * k : BL * (k + 1)],
                                      ps[:, 0:BL])

            pq_ps = psPM.tile([ADIM, BL], f32, tag="psPM")
            for k in range(QC):
                nc.tensor.matmul(
                    pq_ps[:], wq_r[:, 128 * k : 128 * (k + 1)],
                    ahsT[:, BL * k : BL * (k + 1)],
                    start=(k == 0), stop=(k == QC - 1))
            pq_sb = wpool.tile([ADIM, BL], f32, tag="pq_sb")
            nc.scalar.activation(pq_sb[:], pq_ps[:], Act.Tanh, bias=bq_t[:])

            cv_ps = psPM.tile([NFILT, BL], f32, tag="psPM")
            for k in range(TC):
                nc.tensor.matmul(
                    cv_ps[:], wc_r[:, NFILT * k : NFILT * (k + 1)],
                    awcT[:, BL * k : BL * (k + 1)],
                    start=(k == 0), stop=(k == TC - 1))
            cv_sb = wpool.tile([NFILT, BL], f32r, tag="cv_sb")
            nc.scalar.activation(cv_sb[:], cv_ps[:], Act.Tanh)

            paw_ps = psPM.tile([ADIM, BL], f32, tag="psPM")
            nc.tensor.matmul(paw_ps[:], wloc_r[:], cv_sb[:])
            bias_all = wpool.tile([ADIM, BL], f32, tag="bias_all")
            nc.scalar.activation(bias_all[:], paw_ps[:], Act.Tanh)
            nc.vector.tensor_tensor(bias_all[:], bias_all[:], pq_sb[:],
                                    op=mybir.AluOpType.add)

            awt = wpool.tile([128, BL * TC], f32, tag="awt")

            # ---- main pipeline, software-pipelined emission order ----------
            # Engines execute their instruction streams in order, so post(g)
            # (softmax -> aw transpose -> scale -> store) is emitted AFTER
            # compute(g+1); its dependencies resolve while compute(g+1) runs
            # and no engine stalls at group boundaries.
            E_t = {}

            def compute(g):
                off = sum(GROUPS[:g])
                sz = GROUPS[g]
                E = psE.tile([GB, NCH * 512], f32, tag="psE", name=f"E_{g}")
                E_t[g] = E
                for j in range(sz):
                    b = off + j
                    for q in range(NCH):
                        mb = mtpool.tile([128, 4 * ENC], bf16, tag="mem_b",
                                         name=f"mb_{b}_{q}", bufs=3)
                        nc.vector.tensor_copy(
                            mb[:], mem_t[b][:, 2048 * q : 2048 * (q + 1)])
                        mts = []
                        for ecp in range(EC // 2):
                            ps = psT.tile([128, 1024], bf16, tag="psT",
                                          name=f"ps_{b}_{q}_{ecp}")
                            for ec2 in range(2):
                                ec = 2 * ecp + ec2
                                for t4 in range(4):
                                    nc.tensor.transpose(
                                        ps[:, 512 * ec2 + 128 * t4 :
                                           512 * ec2 + 128 * (t4 + 1)],
                                        mb[:, ENC * t4 + 128 * ec :
                                           ENC * t4 + 128 * (ec + 1)],
                                        id_b[:, :])
                            mt = mtpool.tile([128, 1024], bf16, tag="memT",
                                             name=f"mt_{b}_{q}_{ecp}", bufs=4)
                            nc.vector.tensor_copy(mt[:], ps[:])
                            mts.append(mt)
                        pm_ps = psPM.tile([128, 512], f32, tag="psPM",
                                          name=f"pm_ps_{b}_{q}")
                        for ec in range(EC):
                            nc.tensor.matmul(
                                pm_ps[:],
                                wm_b[:, 128 * ec : 128 * (ec + 1)],
                                mts[ec // 2][:, 512 * (ec % 2) :
                                             512 * (ec % 2 + 1)],
                                start=(ec == 0), stop=(ec == EC - 1))
                        pm_sb = epool.tile([128, 512], f32, tag="pm_sb",
                                           name=f"pm_sb_{b}_{q}")
                        nc.scalar.activation(pm_sb[:], pm_ps[:], Act.Tanh,
                                             bias=bm_t[:])
                        e_in = epool.tile([128, 512], f32r, tag="ein",
                                          name=f"e_in_{b}_{q}")
                        nc.scalar.activation(e_in[:], pm_sb[:], Act.Tanh,
                                             bias=bias_all[:, b : b + 1])
                        nc.tensor.matmul(
                            E[0:sz, 512 * q : 512 * (q + 1)],
                            vwz[:, GB * j : GB * j + sz], e_in[:],
                            start=(j == 0), stop=(j == sz - 1),
                            skip_group_check=True)

            def post(g):
                off = sum(GROUPS[:g])
                sz = GROUPS[g]
                E = E_t[g]
                rmax = spool.tile([GB, 1], f32, tag="rmax", name=f"rmax_{g}")
                r1 = spool.tile([GB, 1], f32, tag="r1", name=f"r1_{g}")
                nc.vector.tensor_reduce(rmax[0:sz, :], E[0:sz, 0:512],
                                        mybir.AxisListType.X,
                                        mybir.AluOpType.max)
                nc.vector.tensor_reduce(r1[0:sz, :], E[0:sz, 512:1024],
                                        mybir.AxisListType.X,
                                        mybir.AluOpType.max)
                nc.vector.tensor_tensor(rmax[0:sz, :], rmax[0:sz, :],
                                        r1[0:sz, :],
                                        op=mybir.AluOpType.max)
                nmax = spool.tile([GB, 1], f32, tag="nmax", name=f"nmax_{g}")
                nc.vector.tensor_scalar_mul(nmax[0:sz, :], rmax[0:sz, :], -1.0)
                ex = spool.tile([GB, T], f32, tag="ex", name=f"ex_{g}")
                s0 = spool.tile([GB, 1], f32, tag="s0", name=f"s0_{g}")
                s1 = spool.tile([GB, 1], f32, tag="s1", name=f"s1_{g}")
                nc.scalar.activation(ex[0:sz, 0:512], E[0:sz, 0:512], Act.Exp,
                                     bias=nmax[0:sz, :], accum_out=s0[0:sz, :])
                nc.scalar.activation(ex[0:sz, 512:1024], E[0:sz, 512:1024],
                                     Act.Exp,
                                     bias=nmax[0:sz, :], accum_out=s1[0:sz, :])
                nc.vector.tensor_tensor(s0[0:sz, :], s0[0:sz, :], s1[0:sz, :],
                                        op=mybir.AluOpType.add)
                rs = spool.tile([GB, 1], f32, tag="rs", name=f"rs_{g}")
                nc.vector.reciprocal(rs[0:sz, :], s0[0:sz, :])
                aw_sb = spool.tile([GB, T], f32, tag="aw_sb", name=f"aw_{g}")
                nc.vector.tensor_scalar_mul(aw_sb[0:sz, :], ex[0:sz, :],
                                            rs[0:sz, :])
                nc.gpsimd.dma_start(aw_d[off : off + sz, :], aw_sb[0:sz, :])

                coff = off * TC
                for tcc in range(TC):
                    ps = psT.tile([128, 512], f32, tag="psT",
                                  name=f"awt_ps_{g}_{tcc}")
                    nc.tensor.transpose(
                        ps[:, 0:sz],
                        aw_sb[0:sz, 128 * tcc : 128 * (tcc + 1)],
                        idr[0:sz, 0:sz])
                    nc.vector.tensor_copy(
                        awt[:, coff + sz * tcc : coff + sz * (tcc + 1)],
                        ps[:, 0:sz])

                for j in range(sz):
                    b = off + j
                    for tcc in range(TC):
                        col = coff + sz * tcc + j
                        if tcc < 5:
                            nc.scalar.mul(
                                mem_t[b][:, ENC * tcc : ENC * (tcc + 1)],
                                mem_t[b][:, ENC * tcc : ENC * (tcc + 1)],
                                awt[:, col : col + 1])
                        else:
                            nc.vector.tensor_scalar_mul(
                                mem_t[b][:, ENC * tcc : ENC * (tcc + 1)],
                                mem_t[b][:, ENC * tcc : ENC * (tcc + 1)],
                                awt[:, col : col + 1])
                    for s in range(MEM_DMA_SPLIT):
                        nc.gpsimd.dma_start(
                            ctx_d[b][s * rows : (s + 1) * rows, :]
                            .rearrange("(tc p) e -> p tc e", p=128),
                            mem_t[b][:, s * cols : (s + 1) * cols]
                            .rearrange("p (tc e) -> p tc e", e=ENC))

            compute(0)
            for g in range(1, GRP):
                compute(g)
                post(g - 1)
            post(GRP - 1)

    nc.compile()
    return nc


def _get_nc():
    if "nc" not in _cache:
        _cache["nc"] = _build()
    return _cache["nc"]


def kernel(attention_hidden_state, memory, attention_weights_cat,
           Wq, bq, Wm, bm, conv_w, Wloc, Vw, Vb):
    from concourse.bass_utils import run_bass_kernel_spmd

    mem = np.ascontiguousarray(np.asarray(memory, np.float32))
    ahs = np.ascontiguousarray(
        np.asarray(attention_hidden_state, np.float32).reshape(B, QDIM))
    awc = np.ascontiguousarray(np.asarray(attention_weights_cat, np.float32))
    wq = np.ascontiguousarray(np.asarray(Wq, np.float32))
    wm = np.ascontiguousarray(np.asarray(Wm, np.float32))
    wc = np.ascontiguousarray(np.asarray(conv_w, np.float32)[KSIZE // 2])
    wloc = np.ascontiguousarray(np.asarray(Wloc, np.float32))
    vw = np.ascontiguousarray(np.asarray(Vw, np.float32).reshape(ADIM, 1))
    bqv = np.ascontiguousarray(np.asarray(bq, np.float32).reshape(ADIM, 1))
    bmv = np.ascontiguousarray(np.asarray(bm, np.float32).reshape(ADIM, 1))
    ident = np.eye(128, dtype=np.float32)
    # Vb shifts every energy equally; softmax (and so both outputs) is
    # invariant to it.

    nc = _get_nc()
    in_maps = []
    for c in range(NCORES):
        sl = slice(c * BL, (c + 1) * BL)
        in_maps.append({
            "memory": mem[sl], "ahs": ahs[sl], "awc": awc[sl],
            "wq": wq, "bq": bqv, "wm": wm, "bm": bmv,
            "wc": wc, "wloc": wloc, "vw": vw, "ident": ident,
        })
    res = run_bass_kernel_spmd(nc, in_maps, list(range(NCORES))).results
    ctx = np.concatenate([res[c]["out_ctx"] for c in range(NCORES)], axis=0)
    aw = np.concatenate([res[c]["out_aw"] for c in range(NCORES)], axis=0)
    return ctx, aw
